# revision 12
# baseline (speedup 1.0000x reference)
"""BiMamba block kernel for 8 TRN2 cores (self-contained).

Sharding: core = (direction 2) x (batch 2) x (d_inner half 2).
Device layout: [channels on partitions, time on free dim].
Host does: weight transposes/permutations (local channels first), LN affine
folding into in_proj, time-flip for the backward direction, and the final
gather (sum partials + residual).
"""
from contextlib import ExitStack

import numpy as np
import ml_dtypes

import concourse.bass as bass
import concourse.tile as tile
from concourse import mybir, bacc

F32 = mybir.dt.float32
BF16 = mybir.dt.bfloat16
AF = mybir.ActivationFunctionType
OP = mybir.AluOpType

D_MODEL = 768
D_STATE = 16
D_CONV = 4
D_INNER = 1536
DT_RANK = 48
Bsz, L = 2, 1024
CH = 768                  # local d_inner channels per core
NJ = CH // 128            # 6 local ch tiles
NJF = D_INNER // 128      # 12 full ch tiles
ND = D_MODEL // 128       # 6 model-dim tiles
T = L
H = 512                   # scan-phase time half
EPS = 1e-5

# tuning knobs
SCAN_DVE_N = 16           # walrus rejects TensorScalarPtr on Pool: scans must be DVE
USE_SILU = False          # HW has a Silu table; CoreSim only has Sigmoid

_CACHE = {}


def _build():
    nc = bacc.Bacc()
    xT = nc.dram_tensor("xT", [D_MODEL, T], BF16, kind="ExternalInput")
    in_wT = nc.dram_tensor("in_wT", [D_MODEL, 2304], BF16, kind="ExternalInput")
    W2 = nc.dram_tensor("W2", [2, 2304], BF16, kind="ExternalInput")
    convd = nc.dram_tensor("convd", [128, NJF * D_CONV, 128], BF16,
                           kind="ExternalInput")
    conv_bT = nc.dram_tensor("conv_bT", [128, NJF], F32, kind="ExternalInput")
    xp_wT = nc.dram_tensor("xp_wT", [D_INNER, 80], BF16, kind="ExternalInput")
    dt_wT = nc.dram_tensor("dt_wT", [DT_RANK, CH], BF16, kind="ExternalInput")
    dt_bT = nc.dram_tensor("dt_bT", [128, NJ], F32, kind="ExternalInput")
    A_T = nc.dram_tensor("A_T", [128, NJ, D_STATE], F32, kind="ExternalInput")
    DpT = nc.dram_tensor("DpT", [128, NJ], F32, kind="ExternalInput")
    out_wT = nc.dram_tensor("out_wT", [CH, D_MODEL], BF16, kind="ExternalInput")
    out = nc.dram_tensor("out", [D_MODEL, T], F32, kind="ExternalOutput")

    with ExitStack() as ctx:
        tc = ctx.enter_context(tile.TileContext(nc))
        cp = ctx.enter_context(tc.tile_pool(name="const", bufs=1))
        lnp = ctx.enter_context(tc.tile_pool(name="lnp", bufs=1))
        xtp = ctx.enter_context(tc.tile_pool(name="xtp", bufs=1))
        x2p = ctx.enter_context(tc.tile_pool(name="x2p", bufs=2))
        inwp = ctx.enter_context(tc.tile_pool(name="inwp", bufs=6))
        cvdp = ctx.enter_context(tc.tile_pool(name="cvdp", bufs=3))
        outwp = ctx.enter_context(tc.tile_pool(name="outwp", bufs=6))
        xrawp = ctx.enter_context(tc.tile_pool(name="xraw", bufs=2))
        xcp = ctx.enter_context(tc.tile_pool(name="xcp", bufs=1))
        gatep = ctx.enter_context(tc.tile_pool(name="gatep", bufs=1))
        dltp = ctx.enter_context(tc.tile_pool(name="dlt", bufs=1))
        dlup = ctx.enter_context(tc.tile_pool(name="dlu", bufs=1))
        bcp = ctx.enter_context(tc.tile_pool(name="bc", bufs=1))
        scp = ctx.enter_context(tc.tile_pool(name="sc", bufs=4))
        treep = ctx.enter_context(tc.tile_pool(name="tree", bufs=6))
        pdpp = ctx.enter_context(tc.tile_pool(name="pdp", bufs=2))
        yp = ctx.enter_context(tc.tile_pool(name="yp", bufs=2))
        ygp = ctx.enter_context(tc.tile_pool(name="ygp", bufs=1))
        osbp = ctx.enter_context(tc.tile_pool(name="osb", bufs=2))
        psA = ctx.enter_context(tc.tile_pool(name="psA", bufs=3, space="PSUM"))
        psB = ctx.enter_context(tc.tile_pool(name="psB", bufs=2, space="PSUM"))
        psL = ctx.enter_context(tc.tile_pool(name="psL", bufs=1, space="PSUM"))
        drp = ctx.enter_context(tc.tile_pool(name="dram", bufs=1, space="DRAM"))

        # ---- constants ----
        ones_col = cp.tile([128, 1], BF16)
        nc.vector.memset(ones_col, 1.0)
        A_sb = cp.tile([128, NJ, D_STATE], F32)
        nc.sync.dma_start(out=A_sb, in_=A_T[:, :, :])
        dtb_sb = cp.tile([128, NJ], F32)
        nc.sync.dma_start(out=dtb_sb, in_=dt_bT[:, :])
        Dp_sb = cp.tile([128, NJ], F32)
        nc.sync.dma_start(out=Dp_sb, in_=DpT[:, :])
        cb_sb = cp.tile([128, NJF], F32)
        nc.sync.dma_start(out=cb_sb, in_=conv_bT[:, :])
        W2_sb = cp.tile([2, 2304], BF16)
        nc.sync.dma_start(out=W2_sb, in_=W2[:, :])
        dtw_sb = cp.tile([DT_RANK, CH], BF16)
        nc.sync.dma_start(out=dtw_sb, in_=dt_wT[:, :])
        xpw_sb = cp.tile([128, NJF, 80], BF16)
        nc.sync.dma_start(out=xpw_sb,
                          in_=xp_wT.rearrange("(j p) e -> p j e", p=128))
        eps_sb = cp.tile([1, 1], F32)
        nc.vector.memset(eps_sb, EPS)

        # ---- load x (bf16) ----
        xt = []
        for k in range(ND):
            t_ = xtp.tile([128, T], BF16, tag=f"xt{k}", name=f"xt{k}")
            nc.sync.dma_start(out=t_, in_=xT[k * 128:(k + 1) * 128, :])
            xt.append(t_)

        # ---- layernorm stats via PE ones-matmuls ----
        mean = lnp.tile([1, T], F32)      # sum_x -> mean
        var = lnp.tile([1, T], F32)       # sum_x2 -> e[x2] -> var -> ln -> rstd
        ms = lnp.tile([1, T], F32)        # mean^2 -> mean*rstd
        for c in range(2):
            cs = slice(c * 512, (c + 1) * 512)
            ps_sx = psL.tile([1, 512], F32)
            for k in range(ND):
                nc.tensor.matmul(ps_sx, ones_col[:, :], xt[k][:, cs],
                                 start=(k == 0), stop=(k == ND - 1))
            nc.vector.tensor_copy(out=mean[:, cs], in_=ps_sx)
            ps_s2 = psL.tile([1, 512], F32)
            for k in range(ND):
                x2c = x2p.tile([128, 512], BF16)
                nc.scalar.square(out=x2c, in_=xt[k][:, cs])
                nc.tensor.matmul(ps_s2, ones_col[:, :], x2c[:, :],
                                 start=(k == 0), stop=(k == ND - 1))
            nc.vector.tensor_copy(out=var[:, cs], in_=ps_s2)

        nc.vector.tensor_scalar_mul(out=mean, in0=mean, scalar1=1.0 / D_MODEL)
        nc.vector.tensor_scalar_mul(out=var, in0=var, scalar1=1.0 / D_MODEL)
        nc.vector.tensor_tensor(out=ms, in0=mean, in1=mean, op=OP.mult)
        nc.vector.tensor_tensor(out=var, in0=var, in1=ms, op=OP.subtract)
        nc.scalar.activation(out=var, in_=var, func=AF.Ln, bias=eps_sb[:, :],
                             scale=1.0)
        nc.scalar.activation(out=var, in_=var, func=AF.Exp, bias=0.0, scale=-0.5)
        rstd = var
        nc.vector.tensor_tensor(out=ms, in0=mean, in1=rstd, op=OP.mult)
        W2rhs = cp.tile([2, T], BF16)
        nc.vector.memset(W2rhs, 1.0)
        nc.vector.tensor_copy(out=W2rhs[0:1, :], in_=ms)
        rstd_bc = cp.tile([128, T], BF16)
        rstd_bf = lnp.tile([1, T], BF16)
        nc.vector.tensor_copy(out=rstd_bf, in_=rstd)
        nc.gpsimd.partition_broadcast(rstd_bc, rstd_bf)
        for k in range(ND):
            nc.vector.tensor_tensor(out=xt[k], in0=xt[k], in1=rstd_bc, op=OP.mult)

        # ---- in_proj (e-tiles 0..11 = x_in local-first, 12..17 = z local),
        #      with the depthwise conv fused right after each x e-tile ----
        gate = [None] * NJ
        xc = [None] * NJF
        for i in range(18):
            xr_ = None
            if i < NJF:
                xr_ = xrawp.tile([128, 3 + T], BF16)
                nc.vector.memset(xr_[:, 0:3], 0.0)
            for c in range(2):
                cs = slice(c * 512, (c + 1) * 512)
                ps = psA.tile([128, 512], F32)
                for k in range(ND):
                    wk = inwp.tile([128, 128], BF16, tag="inw")
                    nc.sync.dma_start(
                        out=wk,
                        in_=in_wT[k * 128:(k + 1) * 128, i * 128:(i + 1) * 128])
                    nc.tensor.matmul(ps, wk[:, :], xt[k][:, cs],
                                     start=(k == 0), stop=False)
                nc.tensor.matmul(ps, W2_sb[:, i * 128:(i + 1) * 128],
                                 W2rhs[:, cs], start=False, stop=True)
                if i < NJF:
                    nc.scalar.copy(out=xr_[:, 3 + c * 512:3 + (c + 1) * 512],
                                   in_=ps)
                else:
                    j = i - NJF
                    if gate[j] is None:
                        gate[j] = gatep.tile([128, T], BF16, tag=f"g{j}",
                                             name=f"gate{j}")
                    if USE_SILU:
                        nc.scalar.activation(out=gate[j][:, cs], in_=ps,
                                             func=AF.Silu, bias=0.0, scale=1.0)
                    else:
                        zpre = x2p.tile([128, 512], BF16, tag="zpre")
                        nc.scalar.copy(out=zpre, in_=ps)
                        zsg = x2p.tile([128, 512], BF16, tag="zsg")
                        nc.scalar.activation(out=zsg, in_=ps, func=AF.Sigmoid,
                                             bias=0.0, scale=1.0)
                        nc.vector.tensor_tensor(out=gate[j][:, cs], in0=zpre,
                                                in1=zsg, op=OP.mult)
            if i < NJF:
                # conv for channel tile i consumes xr_ and frees its slot
                j = i
                xc[j] = xcp.tile([128, T], BF16, tag=f"xc{j}", name=f"xc{j}")
                dg = cvdp.tile([128, D_CONV, 128], BF16, tag="convd")
                nc.sync.dma_start(
                    out=dg, in_=convd[:, j * D_CONV:(j + 1) * D_CONV, :])
                for c in range(2):
                    ps = psA.tile([128, 512], F32)
                    for k in range(D_CONV):
                        nc.tensor.matmul(
                            ps, dg[:, k, :],
                            xr_[:, k + c * 512:k + c * 512 + 512],
                            start=(k == 0), stop=(k == D_CONV - 1))
                    ccs = slice(c * 512, (c + 1) * 512)
                    if USE_SILU:
                        nc.scalar.activation(out=xc[j][:, ccs], in_=ps,
                                             func=AF.Silu,
                                             bias=cb_sb[:, j:j + 1], scale=1.0)
                    else:
                        cpre = x2p.tile([128, 512], BF16, tag="cpre")
                        nc.scalar.activation(out=cpre, in_=ps, func=AF.Identity,
                                             bias=cb_sb[:, j:j + 1], scale=1.0)
                        csg = x2p.tile([128, 512], BF16, tag="csg")
                        nc.scalar.activation(out=csg, in_=ps, func=AF.Sigmoid,
                                             bias=cb_sb[:, j:j + 1], scale=1.0)
                        nc.vector.tensor_tensor(out=xc[j][:, ccs], in0=cpre,
                                                in1=csg, op=OP.mult)

        # ---- x_dbl = xp_wT.T @ xc : [80, T] = [dt 48 | B 16 | C 16] ----
        x_dbl = cp.tile([80, T], BF16)
        for c in range(2):
            cs = slice(c * 512, (c + 1) * 512)
            ps = psB.tile([80, 512], F32)
            for j in range(NJF):
                nc.tensor.matmul(ps, xpw_sb[:, j, :], xc[j][:, cs],
                                 start=(j == 0), stop=(j == NJF - 1))
            nc.vector.tensor_copy(out=x_dbl[:, cs], in_=ps)

        # bounce B/C rows through DRAM so they can be partition-broadcast
        scr = drp.tile([32, T], BF16)
        nc.sync.dma_start(out=scr, in_=x_dbl[DT_RANK:80, :])

        # ---- delta_j = softplus(dt_w @ dt + dt_b) (ln(exp(x+b)+1)) ----
        dlt = [None] * NJ
        dlu = [None] * NJ
        hcar = [None] * NJ
        for j in range(NJ):
            dlt[j] = dltp.tile([128, T], BF16, tag=f"dl{j}", name=f"dlt{j}")
            for c in range(2):
                cs = slice(c * 512, (c + 1) * 512)
                ps = psA.tile([128, 512], F32)
                nc.tensor.matmul(ps, dtw_sb[:, j * 128:(j + 1) * 128],
                                 x_dbl[0:DT_RANK, cs], start=True, stop=True)
                nc.scalar.activation(out=dlt[j][:, cs], in_=ps, func=AF.Exp,
                                     bias=dtb_sb[:, j:j + 1], scale=1.0)
            nc.scalar.activation(out=dlt[j], in_=dlt[j], func=AF.Ln, bias=1.0,
                                 scale=1.0)
            dlu[j] = dlup.tile([128, T], BF16, tag=f"du{j}", name=f"dlu{j}")
            nc.vector.tensor_tensor(out=dlu[j], in0=dlt[j], in1=xc[j], op=OP.mult)
            hcar[j] = cp.tile([128, D_STATE], BF16, tag=f"hc{j}", name=f"hcar{j}")

        # ---- scan complex per time-half ----
        ygate = [None] * NJ
        for j in range(NJ):
            ygate[j] = ygp.tile([128, T], BF16, tag=f"yg{j}", name=f"ygate{j}")
        for half in range(2):
            hs = slice(half * H, (half + 1) * H)
            B_bc = bcp.tile([128, D_STATE, H], BF16, tag="Bbc", name="B_bc")
            C_bc = bcp.tile([128, D_STATE, H], BF16, tag="Cbc", name="C_bc")
            nc.gpsimd.dma_start(
                out=B_bc,
                in_=bass.AP(tensor=scr.tensor, offset=scr.offset + half * H,
                            ap=[[0, 128], [T, D_STATE], [1, H]]))
            nc.gpsimd.dma_start(
                out=C_bc,
                in_=bass.AP(tensor=scr.tensor,
                            offset=scr.offset + D_STATE * T + half * H,
                            ap=[[0, 128], [T, D_STATE], [1, H]]))
            for j in range(NJ):
                pDp = pdpp.tile([128, H], BF16, tag="pdp")
                nc.vector.tensor_scalar_mul(out=pDp, in0=xc[j][:, hs],
                                            scalar1=Dp_sb[:, j:j + 1])
                lvl = [None, None, None, None, None]
                for n in range(D_STATE):
                    dA = scp.tile([128, H], BF16, tag="dA")
                    nc.scalar.activation(out=dA, in_=dlt[j][:, hs], func=AF.Exp,
                                         scale=A_sb[:, j, n:n + 1])
                    dBu = scp.tile([128, H], BF16, tag="dBu")
                    nc.vector.tensor_tensor(out=dBu, in0=dlu[j][:, hs],
                                            in1=B_bc[:, n, :], op=OP.mult)
                    h = scp.tile([128, H], BF16, tag="h")
                    eng = nc.vector if n < SCAN_DVE_N else nc.gpsimd
                    ini = 0.0 if half == 0 else hcar[j][:, n:n + 1]
                    eng.tensor_tensor_scan(out=h, data0=dA, data1=dBu,
                                           initial=ini, op0=OP.mult, op1=OP.add)
                    if half == 0:
                        nc.vector.tensor_copy(out=hcar[j][:, n:n + 1],
                                              in_=h[:, H - 1:H])
                    node = treep.tile([128, H], BF16, tag="tr")
                    nc.vector.tensor_tensor(out=node, in0=h, in1=C_bc[:, n, :],
                                            op=OP.mult)
                    lv = 0
                    while lvl[lv] is not None:
                        prev = lvl[lv]
                        lvl[lv] = None
                        nc.vector.tensor_tensor(out=node, in0=prev, in1=node,
                                                op=OP.add)
                        lv += 1
                    lvl[lv] = node
                y = yp.tile([128, H], F32)
                nc.vector.tensor_tensor(out=y, in0=lvl[4], in1=pDp, op=OP.add)
                nc.vector.tensor_tensor(out=ygate[j][:, hs], in0=y,
                                        in1=gate[j][:, hs], op=OP.mult)

        # ---- out_proj ----
        for dm in range(ND):
            ot = osbp.tile([128, T], F32)
            for c in range(2):
                cs = slice(c * 512, (c + 1) * 512)
                ps = psA.tile([128, 512], F32)
                for j in range(NJ):
                    wk = outwp.tile([128, 128], BF16, tag="outw")
                    nc.sync.dma_start(
                        out=wk, in_=out_wT[j * 128:(j + 1) * 128,
                                           dm * 128:(dm + 1) * 128])
                    nc.tensor.matmul(ps, wk[:, :], ygate[j][:, cs],
                                     start=(j == 0), stop=(j == NJ - 1))
                nc.scalar.copy(out=ot[:, cs], in_=ps)
            nc.sync.dma_start(out=out[dm * 128:(dm + 1) * 128, :], in_=ot)

    nc.compile()
    return nc


def get_nc():
    if "nc" not in _CACHE:
        _CACHE["nc"] = _build()
    return _CACHE["nc"]


def prep_core(inputs, direction, b, half):
    """Host-side input prep for one core."""
    f32 = np.float32
    bf16 = ml_dtypes.bfloat16
    p = "f_" if direction == 0 else "b_"
    in_w = np.asarray(inputs[p + "in_w"], f32)
    conv_w = np.asarray(inputs[p + "conv_w"], f32)[:, 0, :]   # [D_INNER, 4]
    conv_b = np.asarray(inputs[p + "conv_b"], f32)
    xp_w = np.asarray(inputs[p + "xp_w"], f32)
    dt_w = np.asarray(inputs[p + "dt_w"], f32)
    dt_b = np.asarray(inputs[p + "dt_b"], f32)
    A = -np.exp(np.asarray(inputs[p + "A_log"], f32))
    Dp = np.asarray(inputs[p + "Dp"], f32)
    out_w = np.asarray(inputs[p + "out_w"], f32)
    gamma = np.asarray(inputs["gamma"], f32)
    beta = np.asarray(inputs["beta"], f32)
    x = np.asarray(inputs["x"], f32)[b]                        # [L, D_MODEL]

    loc = np.arange(half * CH, (half + 1) * CH)
    oth = np.arange((1 - half) * CH, (2 - half) * CH)
    permx = np.concatenate([loc, oth])
    rows = np.concatenate([permx, D_INNER + loc])

    w_eff = in_w * gamma[None, :]
    in_wT = np.ascontiguousarray(w_eff[rows].T).astype(bf16)   # [768, 2304]
    w1 = w_eff[rows].sum(axis=1)
    cvec = in_w[rows] @ beta
    W2 = np.stack([-w1, cvec]).astype(bf16)                    # [2, 2304]

    cw = conv_w[permx]                                         # [1536, 4]
    convd = np.zeros((128, NJF * D_CONV, 128), f32)
    idx = np.arange(128)
    for j in range(NJF):
        for k in range(D_CONV):
            convd[idx, j * D_CONV + k, idx] = cw[j * 128:(j + 1) * 128, k]
    convd = convd.astype(bf16)
    conv_bT = np.ascontiguousarray(conv_b[permx].reshape(NJF, 128).T)
    xp_wT = np.ascontiguousarray(xp_w[:, permx].T).astype(bf16)
    dt_wT = np.ascontiguousarray(dt_w[loc].T).astype(bf16)     # [48, 768]
    dt_bT = np.ascontiguousarray(dt_b[loc].reshape(NJ, 128).T)
    A_T = np.ascontiguousarray(A[loc].reshape(NJ, 128, D_STATE).transpose(1, 0, 2))
    DpT = np.ascontiguousarray(Dp[loc].reshape(NJ, 128).T)
    out_wT = np.ascontiguousarray(out_w[:, loc].T).astype(bf16)

    xTm = np.ascontiguousarray(x.T)                            # [768, 1024]
    if direction == 1:
        xTm = np.ascontiguousarray(xTm[:, ::-1])
    xTm = xTm.astype(bf16)

    return dict(xT=xTm, in_wT=in_wT, W2=W2, convd=convd, conv_bT=conv_bT,
                xp_wT=xp_wT, dt_wT=dt_wT, dt_bT=dt_bT, A_T=A_T, DpT=DpT,
                out_wT=out_wT)


def make_in_maps(inputs):
    maps = []
    for direction in range(2):
        for b in range(Bsz):
            for half in range(2):
                maps.append(prep_core(inputs, direction, b, half))
    return maps


def combine(outs, inputs):
    x = np.asarray(inputs["x"], np.float32)
    y = np.empty((Bsz, L, D_MODEL), np.float32)
    for b in range(Bsz):
        fwd = outs[0 * 4 + b * 2 + 0] + outs[0 * 4 + b * 2 + 1]
        bwd = outs[1 * 4 + b * 2 + 0] + outs[1 * 4 + b * 2 + 1]
        y[b] = x[b] + fwd.T + bwd[:, ::-1].T
    return y


def kernel(**inputs):
    from concourse.bass_utils import run_bass_kernel_spmd
    nc = get_nc()
    in_maps = make_in_maps(inputs)
    res = run_bass_kernel_spmd(nc, in_maps, core_ids=list(range(8)))
    outs = [res.results[i]["out"] for i in range(8)]
    return combine(outs, inputs)
